# revision 23
# baseline (speedup 1.0000x reference)
"""Self-contained TRN2 Bass kernel for nn_DeformConv1d_84739704750225.

kernel(**inputs) takes the FULL unsharded inputs (as produced by
setup_inputs()) and returns the FULL [4, 4096, 512] float32 output.

Internally: data-parallel over (sample, length-half) -> 8 NeuronCores via
run_bass_kernel_spmd. The deformable gather is reformulated as banded
matmuls: per-position window weights W[l, g, j] (j in [0,17)) are scattered
to DRAM as a single fp16 "B-image" plane in the exact [block, span, row]
layout the TensorEngine needs, loaded back with a transposing DMA, and
contracted against fp16 x_proj in one pass. The depthwise conv runs on the
TensorEngine via diagonal weight matrices; LN stats use ones-matmul
reductions; offset/mask nets run in fp16. DMA traffic is split across the
SP and Activation HWDGE rings so the W scatter overlaps the x_proj matmuls.
"""
import sys
sys.path.insert(0, "/opt/trn_rl_repo")
import numpy as np
"""Workarounds for this walrus build's 1-sync-wait-per-instruction limit:

1. TileContext tail drain: put global-clock waits on single-wait SP nops.
2. General post-pass after Tile lowering: any instruction carrying more than
   one sem wait gets preceding same-engine NoOps, one wait each.
"""
import concourse.tile as tile
import concourse.mybir as mybir
from concourse.vector_clock import ScopedClock

MAXW = 1


def _drain_and_barrier(self, tick_clock, wait_clock):
    nc = self.nc
    probe = nc.sync.nop(nofuse=True, hint="tail_wait")
    wait_clock.add_sem_waits(probe.ins, ScopedClock({None: tick_clock.global_clock}))
    waits = list(probe.ins.sync_info.on_wait)
    probe.ins.sync_info.on_wait = waits[:MAXW]
    rest = waits[MAXW:]
    while rest:
        n2 = nc.sync.nop(nofuse=True, hint="tail_wait")
        n2.ins.sync_info = mybir.SyncInfo(on_wait=rest[:MAXW], on_update=[])
        rest = rest[MAXW:]
    nc.sync.drain()
    nc.all_engine_barrier()
    popped = nc._tile_sem_poison_stack.pop()
    assert popped is self._sem_poison
    nc.clear_and_free_semaphores(list(self.sems.allocated().values()))
    nc.all_engine_barrier()


def split_excess_waits(nc, maxw=MAXW):
    """Move all but `maxw` sem-waits of each instruction onto preceding
    same-engine NoOps (program order preserved, so semantics unchanged)."""
    nsplit = 0
    for f in nc.m.functions:
        for blk in f.blocks:
            il = blk.instructions
            i = 0
            while i < len(il):
                inst = il[i]
                si = getattr(inst, "sync_info", None)
                ow = list(si.on_wait) if si is not None else []
                if len(ow) > maxw:
                    si.on_wait = ow[len(ow) - maxw:]
                    extra = ow[:len(ow) - maxw]
                    for j, w in enumerate(extra):
                        n = mybir.InstNoOp(name=f"{inst.name}-ws{j}", ins=[],
                                           outs=[])
                        n.engine = inst.engine
                        n.sync_info = mybir.SyncInfo(on_wait=[w], on_update=[])
                        try:
                            nc.register_instruction(n, overwrite=True)
                        except TypeError:
                            nc.register_instruction(n)
                        il.insert(i, n)
                        i += 1
                    nsplit += 1
                i += 1
    return nsplit


_orig_sched = tile.TileContext.schedule_and_allocate


def _patched_sched(self):
    res = _orig_sched(self)
    split_excess_waits(self.nc)
    return res


tile.TileContext._drain_and_barrier = _drain_and_barrier
tile.TileContext.schedule_and_allocate = _patched_sched



import numpy as np
from contextlib import ExitStack

import bass_rust
import concourse.bass as bass
import concourse.mybir as mybir
import concourse.tile as tile

P = 128
C = 512
CC = 4            # c chunks
G = 4
K = 7
GK = G * K        # 28
J = 17            # band window
L = 4096
LCH = 2048
HALO = 64
LLOC = LCH + 2 * HALO   # 2176
NT = 16           # out l-tiles of 128
NB = 17           # band blocks (= xp tiles), last has 32 rows
NSPAN = 144
COLPAD = 160            # D-plane row stride (128 data + 32 guard cols)
DG = 2448 * COLPAD      # per-g D words
MAGIC = 12582912.0      # 1.5 * 2^23
LN_EPS = 1e-5

f32 = mybir.dt.float32
f32r = mybir.dt.float32r
bf16 = mybir.dt.bfloat16
f16 = mybir.dt.float16
AF = mybir.ActivationFunctionType
OP = mybir.AluOpType


def _ap(t_ap, pairs, offset):
    """Custom access pattern over a tensor's base AP."""
    a = t_ap.copy()
    a.ap = bass_rust.VecI64Pair([list(p) for p in pairs])
    a.offset = offset
    return a


def band_pieces():
    """Per 512-chunk: list of (b, f0, f1, col0). Block b out-span
    l in [128b-16, 128b+128) clipped to [0, LCH), split at 512 boundaries."""
    per_chunk = [[] for _ in range(4)]
    for b in range(NB):
        lo = max(0, 128 * b - 16)
        hi = min(LCH, 128 * b + 128)
        s = lo
        while s < hi:
            e = min(hi, (s // 512 + 1) * 512)
            c = s // 512
            per_chunk[c].append((b, s - (128 * b - 16), e - (128 * b - 16),
                                 s - 512 * c))
            s = e
    return per_chunk


def build_nc():
    nc = bass.Bass()

    def inp(name, shape, dt=f32):
        return nc.dram_tensor(name, shape, dt, kind="ExternalInput")

    xT = inp("xT", [C, LLOC], f32r)
    w_inT = inp("w_inT", [C, C], f32r)    # rows c (contract), cols c_out
    b_in = inp("b_in", [1, C], f32r)
    dwdiag = inp("dwdiag", [P, 12 * P], f32r)  # 12 diag blocks (cc, tap)
    small4 = inp("small4", [P, 4 * CC])   # [dwb | lng | lnb | b_out] cmaj
    w_omT16 = inp("w_omT16", [C, 2 * GK], f16)  # cols: [off 28 | mask 28]
    b_om16 = inp("b_om16", [1, 2 * GK], f16)    # [b_off | b_mask]
    w_outT16 = inp("w_outT16", [C, C], f16)
    vlohi = inp("vlohi", [P, 2 * NT * GK])  # [p, (lo/hi, t, g, k)]
    ones_c = inp("ones_c", [P, 1], f32r)      # 1/512
    ones_cb = inp("ones_cb", [P, 1], bf16)    # 1/512 (for bf16 reductions)
    ones_r = inp("ones_r", [1, P], f32r)      # 1.0
    yT = nc.dram_tensor("yT", [C, LCH], f32, kind="ExternalOutput")

    pieces = band_pieces()
    by_b = [[] for _ in range(NB)]
    for c in range(4):
        for (b, f0, f1, col0) in pieces[c]:
            by_b[b].append((c, f0, f1, col0))
    last_for_chunk = {}
    for b in range(NB):
        for (c, f0, f1, col0) in by_b[b]:
            last_for_chunk[c] = (b, f0)

    with tile.TileContext(nc) as tc, ExitStack() as ctx:
        cpool = ctx.enter_context(tc.tile_pool(name="consts", bufs=1))
        dram = ctx.enter_context(tc.tile_pool(name="dram", bufs=1, space="DRAM"))
        work = ctx.enter_context(tc.tile_pool(name="work", bufs=1))
        tmp2k = ctx.enter_context(tc.tile_pool(name="tmp2k", bufs=2))

        # kernel-lifetime data pools (entered before any scoped pool)
        xdw_pool = ctx.enter_context(tc.tile_pool(name="xdw", bufs=1))
        xdw16_pool = ctx.enter_context(tc.tile_pool(name="xdw16", bufs=1))
        xp_pool = ctx.enter_context(tc.tile_pool(name="xp", bufs=1))
        outT_pool = ctx.enter_context(tc.tile_pool(name="outT", bufs=1))

        # ---------------- input DMAs --------------------------------------
        # SP ring: dwdiag, then xT in 4 column chunks so the conv can start
        # as soon as the first windows land. Everything else rides the Act
        # HWDGE ring in parallel.
        xT_cm = tc.tile_pool(name="xT", bufs=1)
        xT_pool = xT_cm.__enter__()
        xT_sb = xT_pool.tile([P, CC, LLOC], f32r)
        dwd_cm = tc.tile_pool(name="dwd", bufs=1)
        dwd_pool = dwd_cm.__enter__()
        dwdiag_sb = dwd_pool.tile([P, 12, P], f32r)
        nc.sync.dma_start(out=dwdiag_sb[:], in_=dwdiag[:])
        XCOLS = [(0, 640), (640, 1152), (1152, 1664), (1664, 2176)]
        for c0, c1 in XCOLS:
            src = _ap(xT[:], [[LLOC, P], [P * LLOC, CC], [1, c1 - c0]], c0)
            nc.sync.dma_start(out=xT_sb[:, :, c0:c1], in_=src)

        def load_plain(shape, src, tag, dt=f32):
            t = cpool.tile(shape, dt, tag=tag)
            nc.scalar.dma_start(out=t[:], in_=src[:])
            return t

        def load_cmaj(dst, src, ncols):
            # src [C, ncols] -> dst [128, CC, ncols] ; c = cc*128 + p
            src_ap = _ap(src[:], [[ncols, P], [P * ncols, CC], [1, ncols]], 0)
            nc.scalar.dma_start(out=dst[:], in_=src_ap)

        small_sb = load_plain([P, 4 * CC], small4, "small4")
        dwb_col = lambda k: small_sb[:, 0 * CC + k:0 * CC + k + 1]
        lng_col = lambda k: small_sb[:, 1 * CC + k:1 * CC + k + 1]
        lnb_col = lambda k: small_sb[:, 2 * CC + k:2 * CC + k + 1]
        b_out_col = lambda m: small_sb[:, 3 * CC + m:3 * CC + m + 1]
        ones_sb = load_plain([P, 1], ones_c, "ones_c", f32r)
        ones_bf_sb = load_plain([P, 1], ones_cb, "ones_cb", bf16)
        one1_sb = load_plain([1, P], ones_r, "ones_r", f32r)
        vlohi_sb = load_plain([P, 2 * NT * GK], vlohi, "vlohi")
        w_om_sb = cpool.tile([P, CC, 2 * GK], f16)
        load_cmaj(w_om_sb, w_omT16, 2 * GK)
        b_om_sb = load_plain([1, 2 * GK], b_om16, "b_om", f16)
        b_in_sb = load_plain([1, C], b_in, "b_in", f32r)
        w_in_sb = cpool.tile([P, CC, C], f32r)
        load_cmaj(w_in_sb, w_inT, C)
        w_out_sb = cpool.tile([P, CC, C], f16)
        load_cmaj(w_out_sb, w_outT16, C)
        eps_sb = cpool.tile([1, 1], f32)
        nc.gpsimd.memset(eps_sb[:], LN_EPS)
        one1_16 = cpool.tile([1, P], f16)
        nc.gpsimd.memset(one1_16[:], 1.0)
        z1_16 = cpool.tile([1, P], f16)
        nc.gpsimd.memset(z1_16[:], 0.0)
        zrow_16 = cpool.tile([1, C], f16)
        nc.gpsimd.memset(zrow_16[:], 0.0)

        # ---------------- D plane zero (Act ring) -------------------------
        Dpl = dram.tile([G * DG], f16)
        with tc.tile_pool(name="zero", bufs=1) as zpool:
            zt = zpool.tile([P, 3060], f16)
            nc.gpsimd.memset(zt[:], 0.0)
            for i in range(4):
                dst = _ap(Dpl[:], [[3060, P], [1, 3060]], i * DG)
                nc.sync.dma_start(out=dst, in_=zt[:])

        # ---------------- depthwise conv (k=3) on PE ----------------------
        xdw_sb = xdw_pool.tile([P, CC, LCH], bf16)
        with tc.tile_pool(name="psc", bufs=4, space="PSUM") as psc:
            for lc in range(4):
                for k in range(CC):
                    ps = psc.tile([P, 512], f32, tag="psc")
                    for tap in range(3):
                        nc.tensor.matmul(
                            out=ps[:],
                            lhsT=dwdiag_sb[:, 3 * k + tap, :],
                            rhs=xT_sb[:, k, 63 + tap + 512 * lc:
                                      63 + tap + 512 * lc + 512],
                            start=(tap == 0), stop=(tap == 2))
                    nc.scalar.activation(
                        out=xdw_sb[:, k, 512 * lc:512 * lc + 512], in_=ps[:],
                        func=AF.Identity, bias=dwb_col(k), scale=1.0)

        dwd_cm.__exit__(None, None, None)

        # ---------------- LN stats ----------------------------------------
        anorm_cm = tc.tile_pool(name="anorm", bufs=1)
        anorm = anorm_cm.__enter__()
        a_sb = anorm.tile([1, LCH], f16)    # 1/sd
        bn_sb = anorm.tile([1, LCH], f16)   # -mu/sd
        with (tc.tile_pool(name="pst", bufs=2, space="PSUM") as pst,
              tc.tile_pool(name="sqp", bufs=2) as sqp,
              tc.tile_pool(name="smallp", bufs=1) as smallp):
            for lc in range(4):
                sl = slice(512 * lc, 512 * lc + 512)
                pm = pst.tile([1, 512], f32, tag="pmu")
                for k in range(CC):
                    nc.tensor.matmul(
                        out=pm[:], lhsT=ones_bf_sb[:],
                        rhs=xdw_sb[:, k, sl],
                        start=(k == 0), stop=(k == CC - 1))
                pq = pst.tile([1, 512], f32, tag="psq")
                for k in range(CC):
                    sq = sqp.tile([P, 512], f32r, tag="sq")
                    nc.gpsimd.tensor_tensor(out=sq[:], in0=xdw_sb[:, k, sl],
                                            in1=xdw_sb[:, k, sl], op=OP.mult)
                    nc.tensor.matmul(
                        out=pq[:], lhsT=ones_sb[:],
                        rhs=sq[:],
                        start=(k == 0), stop=(k == CC - 1))
                mu_s = smallp.tile([1, 512], f32, tag="mus")
                nc.vector.tensor_copy(out=mu_s[:], in_=pm[:])
                t1 = smallp.tile([1, 512], f32, tag="st1")
                nc.vector.tensor_tensor(out=t1[:], in0=mu_s[:], in1=mu_s[:],
                                        op=OP.mult)
                t2 = smallp.tile([1, 512], f32, tag="st2")
                nc.vector.tensor_tensor(out=t2[:], in0=pq[:], in1=t1[:],
                                        op=OP.subtract)
                t3 = smallp.tile([1, 512], f32, tag="st3")
                nc.scalar.activation(out=t3[:], in_=t2[:], func=AF.Ln,
                                     bias=eps_sb[:])
                t4 = smallp.tile([1, 512], f32, tag="st4")
                nc.scalar.activation(out=t4[:], in_=t3[:], func=AF.Exp,
                                     scale=-0.5)
                nc.vector.tensor_copy(out=a_sb[:, sl], in_=t4[:])
                nc.vector.scalar_tensor_tensor(
                    out=bn_sb[:, sl], in0=mu_s[:], scalar=-1.0,
                    in1=t4[:], op0=OP.mult, op1=OP.mult)

        # ---------------- normalize + GELU --------------------------------
        xdw16 = xdw16_pool.tile([P, CC, LCH], f16)
        a_rep = anorm.tile([P, LCH], bf16)
        bn_rep = anorm.tile([P, LCH], bf16)
        with tc.tile_pool(name="prep", bufs=2, space="PSUM") as prep:
            for lc in range(4):
                sl = slice(512 * lc, 512 * lc + 512)
                pa = prep.tile([P, 512], f32, tag="pa")
                nc.tensor.matmul(out=pa[:], lhsT=one1_16[:],
                                 rhs=a_sb[:, sl],
                                 start=True, stop=True)
                nc.scalar.activation(out=a_rep[:, sl], in_=pa[:], func=AF.Copy)
                pb = prep.tile([P, 512], f32, tag="pb")
                nc.tensor.matmul(out=pb[:], lhsT=one1_16[:],
                                 rhs=bn_sb[:, sl],
                                 start=True, stop=True)
                nc.scalar.activation(out=bn_rep[:, sl], in_=pb[:], func=AF.Copy)

        # x_proj runs in two slices on the PE so it fills the idle windows
        # while the DVE does the normalize and W math.
        xp16 = xp_pool.tile([P, NB, C], f16)
        psx_cm = tc.tile_pool(name="psx", bufs=3, space="PSUM")
        psx = psx_cm.__enter__()

        def xproj_tile(mt):
            M = 128 if mt < 16 else 32
            ps = psx.tile([P, C], f32, tag="psx")
            for k in range(CC):
                nc.tensor.matmul(
                    out=ps[:M, :],
                    lhsT=xT_sb[:, k, 56 + 128 * mt:56 + 128 * mt + M],
                    rhs=w_in_sb[:, k, :],
                    start=(k == 0), stop=False)
            nc.tensor.matmul(
                out=ps[:M, :], lhsT=one1_sb[:1, :M],
                rhs=b_in_sb[:], start=False, stop=True)
            nc.scalar.activation(out=xp16[:M, mt, :], in_=ps[:M, :],
                                 func=AF.Copy)

        for mt in range(8):
            xproj_tile(mt)

        for k in range(CC):
            t1 = tmp2k.tile([P, LCH], bf16, tag="t2k")
            nc.vector.tensor_tensor(
                out=t1[:], in0=xdw_sb[:, k, :], in1=a_rep[:], op=OP.mult)
            t2 = tmp2k.tile([P, LCH], bf16, tag="t2k")
            nc.vector.tensor_tensor(
                out=t2[:], in0=t1[:], in1=bn_rep[:], op=OP.add)
            nc.scalar.activation(out=xdw16[:, k, :], in_=t2[:], func=AF.Gelu,
                                 scale=lng_col(k), bias=lnb_col(k))

        # ---------------- offset/mask nets (fp16) -------------------------
        off_sb = work.tile([P, NT * GK], f32)    # [p, (t, g, k)]
        en_sb = work.tile([P, NT * GK], f32)
        with tc.tile_pool(name="pom", bufs=2, space="PSUM") as pomp:
            for t in range(NT):
                po = pomp.tile([P, 2 * GK], f32, tag="pom")
                for k in range(CC):
                    nc.tensor.matmul(
                        out=po[:],
                        lhsT=xdw16[:, k, 128 * t:128 * t + 128],
                        rhs=w_om_sb[:, k, :],
                        start=(k == 0), stop=False)
                nc.tensor.matmul(
                    out=po[:], lhsT=one1_16[:],
                    rhs=b_om_sb[:], start=False, stop=True)
                nc.vector.tensor_scalar_mul(
                    out=off_sb[:, GK * t:GK * (t + 1)], in0=po[:, 0:GK],
                    scalar1=2.0)
                nc.scalar.activation(out=en_sb[:, GK * t:GK * (t + 1)],
                                     in_=po[:, GK:2 * GK], func=AF.Exp)
        for mt in range(8, NB):
            xproj_tile(mt)
        psx_cm.__exit__(None, None, None)
        anorm_cm.__exit__(None, None, None)
        xT_cm.__exit__(None, None, None)
        # softmax over k
        red_sb = work.tile([P, NT * G], f32)
        en_v = en_sb[:].rearrange("p (tg k) -> p tg k", k=K)
        nc.vector.tensor_reduce(out=red_sb[:], in_=en_v,
                                axis=mybir.AxisListType.X, op=OP.add)
        rln_sb = work.tile([P, NT * G], f32)
        nc.scalar.activation(out=rln_sb[:], in_=red_sb[:], func=AF.Ln)
        rec_sb = work.tile([P, NT * G], f32)
        nc.scalar.activation(out=rec_sb[:], in_=rln_sb[:], func=AF.Exp,
                             scale=-1.0)
        mask_sb = work.tile([P, NT * GK], f32)
        rec_rep = rec_sb[:].unsqueeze(2).broadcast_to([P, NT * G, K])
        nc.vector.tensor_tensor(
            out=mask_sb[:].rearrange("p (tg k) -> p tg k", k=K),
            in0=en_v, in1=rec_rep, op=OP.mult)

        # ---------------- W math ------------------------------------------
        e_sb = work.tile([P, NT * GK], f32)
        nc.vector.tensor_scalar(out=e_sb[:], in0=off_sb[:], scalar1=MAGIC,
                                scalar2=MAGIC, op0=OP.add, op1=OP.subtract)
        gt_sb = work.tile([P, NT * GK], f32)
        nc.vector.tensor_tensor(out=gt_sb[:], in0=e_sb[:], in1=off_sb[:],
                                op=OP.is_gt)
        nc.vector.tensor_tensor(out=e_sb[:], in0=e_sb[:], in1=gt_sb[:],
                                op=OP.subtract)
        frac_sb = work.tile([P, NT * GK], f32)
        nc.vector.tensor_tensor(out=frac_sb[:], in0=off_sb[:], in1=e_sb[:],
                                op=OP.subtract)
        ta_sb = work.tile([P, NT * GK], f32)
        nc.vector.tensor_tensor(out=ta_sb[:], in0=off_sb[:], in1=vlohi_sb[:, :NT * GK],
                                op=OP.is_ge)
        tb_sb = work.tile([P, NT * GK], f32)
        nc.vector.tensor_tensor(out=tb_sb[:], in0=off_sb[:], in1=vlohi_sb[:, NT * GK:],
                                op=OP.is_le)
        nc.vector.tensor_tensor(out=ta_sb[:], in0=ta_sb[:], in1=tb_sb[:],
                                op=OP.mult)
        vm_sb = tb_sb
        nc.vector.tensor_tensor(out=vm_sb[:], in0=ta_sb[:], in1=mask_sb[:],
                                op=OP.mult)
        wgtc_sb = ta_sb
        nc.vector.tensor_tensor(out=wgtc_sb[:], in0=frac_sb[:], in1=vm_sb[:],
                                op=OP.mult)
        wgtf_sb = gt_sb
        nc.vector.tensor_tensor(out=wgtf_sb[:], in0=vm_sb[:], in1=wgtc_sb[:],
                                op=OP.subtract)

        Wf_sb = work.tile([P, NT * G * J], f16)   # [p, (t, g, j)]
        Wc_sb = work.tile([P, NT * G * J], f16)
        nc.gpsimd.memset(Wf_sb[:], 0.0)
        nc.gpsimd.memset(Wc_sb[:], 0.0)
        with tc.tile_pool(name="mf", bufs=2) as mfpool:
            Wf_v = Wf_sb[:].rearrange("p (tg j) -> p tg j", j=J)
            Wc_v = Wc_sb[:].rearrange("p (tg j) -> p tg j", j=J)
            for ev in range(-4, 4):
                mf = mfpool.tile([P, NT * GK], f16, tag="mf")
                nc.vector.scalar_tensor_tensor(
                    out=mf[:], in0=e_sb[:], scalar=float(ev), in1=wgtf_sb[:],
                    op0=OP.is_equal, op1=OP.mult)
                nc.vector.tensor_tensor(
                    out=Wf_v[:, :, 5 + ev:12 + ev],
                    in0=Wf_v[:, :, 5 + ev:12 + ev],
                    in1=mf[:].rearrange("p (tg k) -> p tg k", k=K), op=OP.add)
                mc = mfpool.tile([P, NT * GK], f16, tag="mc")
                nc.vector.scalar_tensor_tensor(
                    out=mc[:], in0=e_sb[:], scalar=float(ev), in1=wgtc_sb[:],
                    op0=OP.is_equal, op1=OP.mult)
                nc.vector.tensor_tensor(
                    out=Wc_v[:, :, 6 + ev:13 + ev],
                    in0=Wc_v[:, :, 6 + ev:13 + ev],
                    in1=mc[:].rearrange("p (tg k) -> p tg k", k=K), op=OP.add)
        nc.vector.tensor_tensor(out=Wf_sb[:], in0=Wf_sb[:], in1=Wc_sb[:],
                                op=OP.add)

        # ---------------- W -> D-plane scatter (Act ring) -----------------
        Wv = Wf_sb[:].rearrange("p (t g j) -> p t g j", g=G, j=J)
        for g in range(G):
            base = g * DG
            dst = _ap(Dpl[:], [[161, P], [23040, NT], [1, J]], base + 2560)
            nc.scalar.dma_start(out=dst, in_=Wv[:, :, g, :])
            dst2 = _ap(Dpl[:], [[161, 16], [23040, NT], [1, J]],
                       base + 4992 + 161 * 112)
            nc.scalar.dma_start(out=dst2, in_=Wv[112:128, :, g, :])

        # ---------------- band matmuls + y projection ---------------------
        outT_sb = outT_pool.tile([P, G, LCH], f16)
        with (tc.tile_pool(name="band", bufs=2) as bpool,
              tc.tile_pool(name="pband", bufs=8, space="PSUM") as pbp):
            for g in range(G):
                B16 = bpool.tile([P, NB * NSPAN], f16, tag="bimg")
                nc.sync.dma_start(
                    out=B16[:],
                    in_=_ap(Dpl[:], [[COLPAD, NB * NSPAN], [1, P]], g * DG),
                    transpose=True)
                pbs = []
                for c in range(4):
                    pb = pbp.tile([P, 512], f32, tag="pband")
                    nc.tensor.matmul(out=pb[:], lhsT=z1_16[:],
                                     rhs=zrow_16[:], start=True, stop=False)
                    pbs.append(pb)
                for b in range(NB):
                    kb = 128 if b < 16 else 32
                    for (c, f0, f1, col0) in by_b[b]:
                        nc.tensor.matmul(
                            out=pbs[c][:, col0:col0 + (f1 - f0)],
                            lhsT=xp16[:kb, b, 128 * g:128 * g + 128],
                            rhs=B16[:kb, 144 * b + f0:144 * b + f1],
                            start=False,
                            stop=(last_for_chunk[c] == (b, f0)))
                for c in range(4):
                    nc.scalar.activation(
                        out=outT_sb[:, g, 512 * c:512 * c + 512],
                        in_=pbs[c][:], func=AF.Copy)

        # ---------------- y projection ------------------------------------
        with (tc.tile_pool(name="y", bufs=3) as ypool,
              tc.tile_pool(name="py", bufs=4, space="PSUM") as pyp):
            for c in range(4):
                for m in range(CC):
                    py = pyp.tile([P, 512], f32, tag="py")
                    for k in range(CC):
                        nc.tensor.matmul(
                            out=py[:],
                            lhsT=w_out_sb[:, k, 128 * m:128 * m + 128],
                            rhs=outT_sb[:, k, 512 * c:512 * c + 512],
                            start=(k == 0), stop=(k == CC - 1))
                    ysb = ypool.tile([P, 512], f32, tag="ysb")
                    nc.scalar.activation(out=ysb[:], in_=py[:],
                                         func=AF.Identity,
                                         bias=b_out_col(m),
                                         scale=1.0)
                    ydst = _ap(yT[:], [[LCH, P], [1, 512]],
                               128 * m * LCH + 512 * c)
                    yeng = nc.sync if (c * 4 + m) % 2 == 0 else nc.scalar
                    yeng.dma_start(out=ydst, in_=ysb[:])
    return nc


# ---------------- host-side helpers ----------------

def make_core_inputs(inputs, core):
    """Build the per-core input dict from the full problem inputs."""
    n, h = core // 2, core % 2
    start = h * LCH
    x = np.asarray(inputs["x"], np.float32)
    xpad = np.zeros((L + 2 * HALO, C), np.float32)
    xpad[HALO:HALO + L] = x[n]
    xT = np.ascontiguousarray(xpad[start:start + LLOC].T)

    def cmaj(a):  # [C] -> [128, CC] with c = cc*128 + p
        return np.ascontiguousarray(np.asarray(a, np.float32).reshape(CC, P).T)

    dw = np.asarray(inputs["dw_w"], np.float32)[:, 0, :]   # [C, 3]
    dwdiag = np.zeros((P, 12, P), np.float32)
    rng = np.arange(P)
    for cc in range(CC):
        for tap in range(3):
            dwdiag[rng, 3 * cc + tap, rng] = dw[cc * P + rng, tap]

    pos = start + np.arange(LCH)
    kk = np.arange(K)
    pos_ptk = pos.reshape(NT, P).T[:, :, None, None]       # [p, t, 1, 1]
    ones = np.ones((P, NT, G, K), np.float32)
    vlo = (3 - kk[None, None, None, :] - pos_ptk) * ones
    vhi = (L + 2 - kk[None, None, None, :] - pos_ptk) * ones

    f = np.float32
    h16 = np.float16
    small4v = np.concatenate(
        [cmaj(inputs["dw_b"]), cmaj(inputs["ln_g"]),
         cmaj(inputs["ln_b"]), cmaj(inputs["b_out"])], 1)
    vlohiv = np.concatenate(
        [vlo.reshape(P, NT * GK), vhi.reshape(P, NT * GK)], 1)
    return {
        "xT": xT.astype(f),
        "w_inT": np.ascontiguousarray(np.asarray(inputs["w_in"]).T).astype(f),
        "b_in": np.asarray(inputs["b_in"]).reshape(1, C).astype(f),
        "dwdiag": np.ascontiguousarray(dwdiag.reshape(P, 12 * P)).astype(f),
        "small4": np.ascontiguousarray(small4v).astype(f),
        "w_omT16": np.ascontiguousarray(np.concatenate(
            [np.asarray(inputs["w_off"]).T, np.asarray(inputs["w_mask"]).T],
            1)).astype(h16),
        "b_om16": np.concatenate([np.asarray(inputs["b_off"]),
                                  np.asarray(inputs["b_mask"])]).reshape(
                                      1, 2 * GK).astype(h16),
        "w_outT16": np.ascontiguousarray(
            np.asarray(inputs["w_out"]).T).astype(h16),
        "vlohi": np.ascontiguousarray(vlohiv).astype(f),
        "ones_c": np.full((P, 1), 1.0 / C, f),
        "ones_cb": _bf16_full((P, 1), 1.0 / C),
        "ones_r": np.ones((1, P), f),
    }


def _bf16_full(shape, val):
    import ml_dtypes
    return np.full(shape, val, ml_dtypes.bfloat16)


def assemble(results):
    """results: list of 8 dicts with 'yT' [C, LCH] -> full [4, L, C]."""
    out = np.zeros((4, L, C), np.float32)
    for core in range(8):
        n, h = core // 2, core % 2
        out[n, h * LCH:(h + 1) * LCH] = results[core]["yT"].T
    return out


_NC_CACHE = {}


def kernel(**inputs):
    """Full-problem entry point. inputs keyed as in setup_inputs()."""
    from concourse.bass_utils import run_bass_kernel_spmd
    if "nc" not in _NC_CACHE:
        _NC_CACHE["nc"] = build_nc()
    nc = _NC_CACHE["nc"]
    in_maps = [make_core_inputs(inputs, core) for core in range(8)]
    res = run_bass_kernel_spmd(nc, in_maps, core_ids=list(range(8)))
    return assemble(res.results)


# revision 26
# speedup vs baseline: 1.1504x; 1.1504x over previous
"""Self-contained TRN2 Bass kernel for nn_DeformConv1d_84739704750225.

kernel(**inputs) takes the FULL unsharded inputs (as produced by
setup_inputs()) and returns the FULL [4, 4096, 512] float32 output.

Internally: data-parallel over (sample, length-half) -> 8 NeuronCores via
run_bass_kernel_spmd. The deformable gather is reformulated as banded
matmuls: per-position window weights W[l, g, j] (j in [0,17)) are scattered
to DRAM as a single fp16 "B-image" plane in the exact [block, span, row]
layout the TensorEngine needs, loaded back with a transposing DMA, and
contracted against fp16 x_proj in one pass. The depthwise conv runs on the
TensorEngine via diagonal weight matrices; LN stats use ones-matmul
reductions; offset/mask nets run in fp16. DMA traffic is split across the
SP and Activation HWDGE rings so the W scatter overlaps the x_proj matmuls.
"""
import sys
sys.path.insert(0, "/opt/trn_rl_repo")
import numpy as np
"""Workarounds for this walrus build's 1-sync-wait-per-instruction limit:

1. TileContext tail drain: put global-clock waits on single-wait SP nops.
2. General post-pass after Tile lowering: any instruction carrying more than
   one sem wait gets preceding same-engine NoOps, one wait each.
"""
import concourse.tile as tile
import concourse.mybir as mybir
from concourse.vector_clock import ScopedClock

MAXW = 1


def _drain_and_barrier(self, tick_clock, wait_clock):
    nc = self.nc
    probe = nc.sync.nop(nofuse=True, hint="tail_wait")
    wait_clock.add_sem_waits(probe.ins, ScopedClock({None: tick_clock.global_clock}))
    waits = list(probe.ins.sync_info.on_wait)
    probe.ins.sync_info.on_wait = waits[:MAXW]
    rest = waits[MAXW:]
    while rest:
        n2 = nc.sync.nop(nofuse=True, hint="tail_wait")
        n2.ins.sync_info = mybir.SyncInfo(on_wait=rest[:MAXW], on_update=[])
        rest = rest[MAXW:]
    nc.sync.drain()
    nc.all_engine_barrier()
    popped = nc._tile_sem_poison_stack.pop()
    assert popped is self._sem_poison
    nc.clear_and_free_semaphores(list(self.sems.allocated().values()))
    nc.all_engine_barrier()


def split_excess_waits(nc, maxw=MAXW):
    """Move all but `maxw` sem-waits of each instruction onto preceding
    same-engine NoOps (program order preserved, so semantics unchanged)."""
    nsplit = 0
    for f in nc.m.functions:
        for blk in f.blocks:
            il = blk.instructions
            i = 0
            while i < len(il):
                inst = il[i]
                si = getattr(inst, "sync_info", None)
                ow = list(si.on_wait) if si is not None else []
                if len(ow) > maxw:
                    si.on_wait = ow[len(ow) - maxw:]
                    extra = ow[:len(ow) - maxw]
                    for j, w in enumerate(extra):
                        n = mybir.InstNoOp(name=f"{inst.name}-ws{j}", ins=[],
                                           outs=[])
                        n.engine = inst.engine
                        n.sync_info = mybir.SyncInfo(on_wait=[w], on_update=[])
                        try:
                            nc.register_instruction(n, overwrite=True)
                        except TypeError:
                            nc.register_instruction(n)
                        il.insert(i, n)
                        i += 1
                    nsplit += 1
                i += 1
    return nsplit


_orig_sched = tile.TileContext.schedule_and_allocate


def _patched_sched(self):
    res = _orig_sched(self)
    split_excess_waits(self.nc)
    return res


tile.TileContext._drain_and_barrier = _drain_and_barrier
tile.TileContext.schedule_and_allocate = _patched_sched



import numpy as np
from contextlib import ExitStack

import bass_rust
import concourse.bass as bass
import concourse.mybir as mybir
import concourse.tile as tile

P = 128
C = 512
CC = 4            # c chunks
G = 4
K = 7
GK = G * K        # 28
J = 17            # band window
L = 4096
LCH = 2048
HALO = 64
LLOC = LCH + 2 * HALO   # 2176
NT = 16           # out l-tiles of 128
NB = 17           # band blocks (= xp tiles), last has 32 rows
NSPAN = 144
COLPAD = 160            # D-plane row stride (128 data + 32 guard cols)
DG = 2448 * COLPAD      # per-g D words
MAGIC = 12582912.0      # 1.5 * 2^23
LN_EPS = 1e-5

f32 = mybir.dt.float32
f32r = mybir.dt.float32r
bf16 = mybir.dt.bfloat16
f16 = mybir.dt.float16
AF = mybir.ActivationFunctionType
OP = mybir.AluOpType


def _ap(t_ap, pairs, offset):
    """Custom access pattern over a tensor's base AP."""
    a = t_ap.copy()
    a.ap = bass_rust.VecI64Pair([list(p) for p in pairs])
    a.offset = offset
    return a


def band_pieces():
    """Per 512-chunk: list of (b, f0, f1, col0). Block b out-span
    l in [128b-16, 128b+128) clipped to [0, LCH), split at 512 boundaries."""
    per_chunk = [[] for _ in range(4)]
    for b in range(NB):
        lo = max(0, 128 * b - 16)
        hi = min(LCH, 128 * b + 128)
        s = lo
        while s < hi:
            e = min(hi, (s // 512 + 1) * 512)
            c = s // 512
            per_chunk[c].append((b, s - (128 * b - 16), e - (128 * b - 16),
                                 s - 512 * c))
            s = e
    return per_chunk


def build_nc():
    nc = bass.Bass()

    def inp(name, shape, dt=f32):
        return nc.dram_tensor(name, shape, dt, kind="ExternalInput")

    xT = inp("xT", [C, LLOC], f16)
    w_inT = inp("w_inT", [C, C], f16)     # rows c (contract), cols c_out
    b_in = inp("b_in", [1, C], f16)
    dwdiag = inp("dwdiag", [P, 12 * P], f16)  # 12 diag blocks (cc, tap)
    small4 = inp("small4", [P, 4 * CC])   # [dwb | lng | lnb | b_out] cmaj
    w_omT16 = inp("w_omT16", [C, 2 * GK], f16)  # cols: [off 28 | mask 28]
    b_om16 = inp("b_om16", [1, 2 * GK], f16)    # [b_off | b_mask]
    w_outT16 = inp("w_outT16", [C, C], f16)
    vlohi = inp("vlohi", [P, 2 * NT * GK])  # [p, (lo/hi, t, g, k)]
    ones_cb = inp("ones_cb", [P, 1], bf16)    # 1/512 (for bf16 reductions)
    yT = nc.dram_tensor("yT", [C, LCH], f32, kind="ExternalOutput")

    pieces = band_pieces()
    by_b = [[] for _ in range(NB)]
    for c in range(4):
        for (b, f0, f1, col0) in pieces[c]:
            by_b[b].append((c, f0, f1, col0))
    last_for_chunk = {}
    for b in range(NB):
        for (c, f0, f1, col0) in by_b[b]:
            last_for_chunk[c] = (b, f0)

    with tile.TileContext(nc) as tc, ExitStack() as ctx:
        cpool = ctx.enter_context(tc.tile_pool(name="consts", bufs=1))
        dram = ctx.enter_context(tc.tile_pool(name="dram", bufs=1, space="DRAM"))
        work = ctx.enter_context(tc.tile_pool(name="work", bufs=1))
        tmp2k = ctx.enter_context(tc.tile_pool(name="tmp2k", bufs=2))

        # kernel-lifetime data pools (entered before any scoped pool)
        xdw_pool = ctx.enter_context(tc.tile_pool(name="xdw", bufs=1))
        xdw16_pool = ctx.enter_context(tc.tile_pool(name="xdw16", bufs=1))
        xp_pool = ctx.enter_context(tc.tile_pool(name="xp", bufs=1))
        outT_pool = ctx.enter_context(tc.tile_pool(name="outT", bufs=1))

        # ---------------- input DMAs --------------------------------------
        # SP ring: dwdiag, then xT in 4 column chunks so the conv can start
        # as soon as the first windows land. Everything else rides the Act
        # HWDGE ring in parallel.
        xT_cm = tc.tile_pool(name="xT", bufs=1)
        xT_pool = xT_cm.__enter__()
        xT_sb = xT_pool.tile([P, CC, LLOC], f16)
        dwd_cm = tc.tile_pool(name="dwd", bufs=1)
        dwd_pool = dwd_cm.__enter__()
        dwdiag_sb = dwd_pool.tile([P, 12, P], f16)
        nc.sync.dma_start(out=dwdiag_sb[:], in_=dwdiag[:])
        XCOLS = [(0, 640), (640, 1152), (1152, 1664), (1664, 2176)]
        for c0, c1 in XCOLS:
            src = _ap(xT[:], [[LLOC, P], [P * LLOC, CC], [1, c1 - c0]], c0)
            nc.sync.dma_start(out=xT_sb[:, :, c0:c1], in_=src)

        def load_plain(shape, src, tag, dt=f32):
            t = cpool.tile(shape, dt, tag=tag)
            nc.scalar.dma_start(out=t[:], in_=src[:])
            return t

        def load_cmaj(dst, src, ncols):
            # src [C, ncols] -> dst [128, CC, ncols] ; c = cc*128 + p
            src_ap = _ap(src[:], [[ncols, P], [P * ncols, CC], [1, ncols]], 0)
            nc.scalar.dma_start(out=dst[:], in_=src_ap)

        small_sb = load_plain([P, 4 * CC], small4, "small4")
        dwb_col = lambda k: small_sb[:, 0 * CC + k:0 * CC + k + 1]
        lng_col = lambda k: small_sb[:, 1 * CC + k:1 * CC + k + 1]
        lnb_col = lambda k: small_sb[:, 2 * CC + k:2 * CC + k + 1]
        b_out_col = lambda m: small_sb[:, 3 * CC + m:3 * CC + m + 1]
        ones_bf_sb = load_plain([P, 1], ones_cb, "ones_cb", bf16)
        vlohi_sb = load_plain([P, 2 * NT * GK], vlohi, "vlohi")
        w_om_sb = cpool.tile([P, CC, 2 * GK], f16)
        load_cmaj(w_om_sb, w_omT16, 2 * GK)
        b_om_sb = load_plain([1, 2 * GK], b_om16, "b_om", f16)
        b_in_sb = load_plain([1, C], b_in, "b_in", f16)
        w_in_sb = cpool.tile([P, CC, C], f16)
        load_cmaj(w_in_sb, w_inT, C)
        w_out_sb = cpool.tile([P, CC, C], f16)
        load_cmaj(w_out_sb, w_outT16, C)
        eps_sb = cpool.tile([1, 1], f32)
        nc.gpsimd.memset(eps_sb[:], LN_EPS)
        one1_16 = cpool.tile([1, P], f16)
        nc.gpsimd.memset(one1_16[:], 1.0)
        z1_16 = cpool.tile([1, P], f16)
        nc.gpsimd.memset(z1_16[:], 0.0)
        zrow_16 = cpool.tile([1, C], f16)
        nc.gpsimd.memset(zrow_16[:], 0.0)

        # ---------------- D plane zero (SP ring) --------------------------
        Dpls = [dram.tile([DG], f16, name="dpl%d" % g, tag="dpl%d" % g)
                for g in range(G)]
        with tc.tile_pool(name="zero", bufs=1) as zpool:
            zt = zpool.tile([P, 3060], f16)
            nc.gpsimd.memset(zt[:], 0.0)
            for g in range(G):
                dst = _ap(Dpls[g][:], [[3060, P], [1, 3060]], 0)
                nc.sync.dma_start(out=dst, in_=zt[:])

        # ---------------- depthwise conv (k=3) on PE ----------------------
        xdw_sb = xdw_pool.tile([P, CC, LCH], bf16)
        with tc.tile_pool(name="psc", bufs=4, space="PSUM") as psc:
            for lc in range(4):
                for k in range(CC):
                    ps = psc.tile([P, 512], f32, tag="psc")
                    for tap in range(3):
                        nc.tensor.matmul(
                            out=ps[:],
                            lhsT=dwdiag_sb[:, 3 * k + tap, :],
                            rhs=xT_sb[:, k, 63 + tap + 512 * lc:
                                      63 + tap + 512 * lc + 512],
                            start=(tap == 0), stop=(tap == 2))
                    nc.scalar.activation(
                        out=xdw_sb[:, k, 512 * lc:512 * lc + 512], in_=ps[:],
                        func=AF.Identity, bias=dwb_col(k), scale=1.0)

        dwd_cm.__exit__(None, None, None)

        # ---------------- LN stats ----------------------------------------
        anorm_cm = tc.tile_pool(name="anorm", bufs=1)
        anorm = anorm_cm.__enter__()
        a_sb = anorm.tile([1, LCH], f16)    # 1/sd
        bn_sb = anorm.tile([1, LCH], f16)   # -mu/sd
        with (tc.tile_pool(name="pst", bufs=2, space="PSUM") as pst,
              tc.tile_pool(name="sqp", bufs=2) as sqp,
              tc.tile_pool(name="smallp", bufs=1) as smallp):
            for lc in range(4):
                sl = slice(512 * lc, 512 * lc + 512)
                pm = pst.tile([1, 512], f32, tag="pmu")
                for k in range(CC):
                    nc.tensor.matmul(
                        out=pm[:], lhsT=ones_bf_sb[:],
                        rhs=xdw_sb[:, k, sl],
                        start=(k == 0), stop=(k == CC - 1))
                pq = pst.tile([1, 512], f32, tag="psq")
                for k in range(CC):
                    sq = sqp.tile([P, 512], bf16, tag="sq")
                    nc.scalar.activation(out=sq[:], in_=xdw_sb[:, k, sl],
                                         func=AF.Square)
                    nc.tensor.matmul(
                        out=pq[:], lhsT=ones_bf_sb[:],
                        rhs=sq[:],
                        start=(k == 0), stop=(k == CC - 1))
                mu_s = smallp.tile([1, 512], f32, tag="mus")
                nc.vector.tensor_copy(out=mu_s[:], in_=pm[:])
                t1 = smallp.tile([1, 512], f32, tag="st1")
                nc.vector.tensor_tensor(out=t1[:], in0=mu_s[:], in1=mu_s[:],
                                        op=OP.mult)
                t2 = smallp.tile([1, 512], f32, tag="st2")
                nc.vector.tensor_tensor(out=t2[:], in0=pq[:], in1=t1[:],
                                        op=OP.subtract)
                t3 = smallp.tile([1, 512], f32, tag="st3")
                nc.scalar.activation(out=t3[:], in_=t2[:], func=AF.Ln,
                                     bias=eps_sb[:])
                t4 = smallp.tile([1, 512], f32, tag="st4")
                nc.scalar.activation(out=t4[:], in_=t3[:], func=AF.Exp,
                                     scale=-0.5)
                nc.vector.tensor_copy(out=a_sb[:, sl], in_=t4[:])
                nc.vector.scalar_tensor_tensor(
                    out=bn_sb[:, sl], in0=mu_s[:], scalar=-1.0,
                    in1=t4[:], op0=OP.mult, op1=OP.mult)

        # ---------------- normalize + GELU --------------------------------
        xdw16 = xdw16_pool.tile([P, CC, LCH], f16)
        a_rep = anorm.tile([P, LCH], bf16)
        bn_rep = anorm.tile([P, LCH], bf16)
        with tc.tile_pool(name="prep", bufs=2, space="PSUM") as prep:
            for lc in range(4):
                sl = slice(512 * lc, 512 * lc + 512)
                pa = prep.tile([P, 512], f32, tag="pa")
                nc.tensor.matmul(out=pa[:], lhsT=one1_16[:],
                                 rhs=a_sb[:, sl],
                                 start=True, stop=True)
                nc.scalar.activation(out=a_rep[:, sl], in_=pa[:], func=AF.Copy)
                pb = prep.tile([P, 512], f32, tag="pb")
                nc.tensor.matmul(out=pb[:], lhsT=one1_16[:],
                                 rhs=bn_sb[:, sl],
                                 start=True, stop=True)
                nc.scalar.activation(out=bn_rep[:, sl], in_=pb[:], func=AF.Copy)

        # x_proj runs in two slices on the PE so it fills the idle windows
        # while the DVE does the normalize and W math.
        xp16 = xp_pool.tile([P, NB, C], f16)
        psx_cm = tc.tile_pool(name="psx", bufs=3, space="PSUM")
        psx = psx_cm.__enter__()

        def xproj_tile(mt):
            M = 128 if mt < 16 else 32
            ps = psx.tile([P, C], f32, tag="psx")
            for k in range(CC):
                nc.tensor.matmul(
                    out=ps[:M, :],
                    lhsT=xT_sb[:, k, 56 + 128 * mt:56 + 128 * mt + M],
                    rhs=w_in_sb[:, k, :],
                    start=(k == 0), stop=False)
            nc.tensor.matmul(
                out=ps[:M, :], lhsT=one1_16[:1, :M],
                rhs=b_in_sb[:], start=False, stop=True)
            nc.scalar.activation(out=xp16[:M, mt, :], in_=ps[:M, :],
                                 func=AF.Copy)

        for mt in range(4):
            xproj_tile(mt)

        for k in range(CC):
            t1 = tmp2k.tile([P, LCH], bf16, tag="t2k")
            nc.vector.tensor_tensor(
                out=t1[:], in0=xdw_sb[:, k, :], in1=a_rep[:], op=OP.mult)
            t2 = tmp2k.tile([P, LCH], bf16, tag="t2k")
            nc.vector.tensor_tensor(
                out=t2[:], in0=t1[:], in1=bn_rep[:], op=OP.add)
            nc.scalar.activation(out=xdw16[:, k, :], in_=t2[:], func=AF.Gelu,
                                 scale=lng_col(k), bias=lnb_col(k))

        # ---------------- offset/mask nets (fp16) -------------------------
        off_sb = work.tile([P, NT * GK], f32)    # [p, (t, g, k)]
        en_sb = work.tile([P, NT * GK], f32)
        with tc.tile_pool(name="pom", bufs=2, space="PSUM") as pomp:
            for t in range(NT):
                po = pomp.tile([P, 2 * GK], f32, tag="pom")
                for k in range(CC):
                    nc.tensor.matmul(
                        out=po[:],
                        lhsT=xdw16[:, k, 128 * t:128 * t + 128],
                        rhs=w_om_sb[:, k, :],
                        start=(k == 0), stop=False)
                nc.tensor.matmul(
                    out=po[:], lhsT=one1_16[:],
                    rhs=b_om_sb[:], start=False, stop=True)
                nc.vector.tensor_scalar_mul(
                    out=off_sb[:, GK * t:GK * (t + 1)], in0=po[:, 0:GK],
                    scalar1=2.0)
                nc.scalar.activation(out=en_sb[:, GK * t:GK * (t + 1)],
                                     in_=po[:, GK:2 * GK], func=AF.Exp)
        for mt in range(4, NB):
            xproj_tile(mt)
        psx_cm.__exit__(None, None, None)
        anorm_cm.__exit__(None, None, None)
        xT_cm.__exit__(None, None, None)
        # softmax over k
        red_sb = work.tile([P, NT * G], f32)
        en_v = en_sb[:].rearrange("p (tg k) -> p tg k", k=K)
        nc.vector.tensor_reduce(out=red_sb[:], in_=en_v,
                                axis=mybir.AxisListType.X, op=OP.add)
        rln_sb = work.tile([P, NT * G], f32)
        nc.scalar.activation(out=rln_sb[:], in_=red_sb[:], func=AF.Ln)
        rec_sb = work.tile([P, NT * G], f32)
        nc.scalar.activation(out=rec_sb[:], in_=rln_sb[:], func=AF.Exp,
                             scale=-1.0)
        mask_sb = work.tile([P, NT * GK], f32)
        rec_rep = rec_sb[:].unsqueeze(2).broadcast_to([P, NT * G, K])
        nc.vector.tensor_tensor(
            out=mask_sb[:].rearrange("p (tg k) -> p tg k", k=K),
            in0=en_v, in1=rec_rep, op=OP.mult)

        # ---------------- W math ------------------------------------------
        e_sb = work.tile([P, NT * GK], f32)
        nc.vector.tensor_scalar(out=e_sb[:], in0=off_sb[:], scalar1=MAGIC,
                                scalar2=MAGIC, op0=OP.add, op1=OP.subtract)
        gt_sb = work.tile([P, NT * GK], f32)
        nc.vector.tensor_tensor(out=gt_sb[:], in0=e_sb[:], in1=off_sb[:],
                                op=OP.is_gt)
        nc.vector.tensor_tensor(out=e_sb[:], in0=e_sb[:], in1=gt_sb[:],
                                op=OP.subtract)
        frac_sb = work.tile([P, NT * GK], f32)
        nc.vector.tensor_tensor(out=frac_sb[:], in0=off_sb[:], in1=e_sb[:],
                                op=OP.subtract)
        ta_sb = work.tile([P, NT * GK], f32)
        nc.vector.tensor_tensor(out=ta_sb[:], in0=off_sb[:], in1=vlohi_sb[:, :NT * GK],
                                op=OP.is_ge)
        tb_sb = work.tile([P, NT * GK], f32)
        nc.vector.tensor_tensor(out=tb_sb[:], in0=off_sb[:], in1=vlohi_sb[:, NT * GK:],
                                op=OP.is_le)
        nc.vector.tensor_tensor(out=ta_sb[:], in0=ta_sb[:], in1=tb_sb[:],
                                op=OP.mult)
        vm_sb = tb_sb
        nc.vector.tensor_tensor(out=vm_sb[:], in0=ta_sb[:], in1=mask_sb[:],
                                op=OP.mult)
        wgtc_sb = ta_sb
        nc.vector.tensor_tensor(out=wgtc_sb[:], in0=frac_sb[:], in1=vm_sb[:],
                                op=OP.mult)
        wgtf_sb = gt_sb
        nc.vector.tensor_tensor(out=wgtf_sb[:], in0=vm_sb[:], in1=wgtc_sb[:],
                                op=OP.subtract)

        Wf_sb = work.tile([P, NT * G * J], f16)   # [p, (t, g, j)]
        Wc_sb = work.tile([P, NT * G * J], f16)
        nc.gpsimd.memset(Wf_sb[:], 0.0)
        nc.gpsimd.memset(Wc_sb[:], 0.0)
        with tc.tile_pool(name="mf", bufs=2) as mfpool:
            Wf_v = Wf_sb[:].rearrange("p (tg j) -> p tg j", j=J)
            Wc_v = Wc_sb[:].rearrange("p (tg j) -> p tg j", j=J)
            for ev in range(-4, 4):
                mf = mfpool.tile([P, NT * GK], f16, tag="mf")
                nc.vector.scalar_tensor_tensor(
                    out=mf[:], in0=e_sb[:], scalar=float(ev), in1=wgtf_sb[:],
                    op0=OP.is_equal, op1=OP.mult)
                nc.vector.tensor_tensor(
                    out=Wf_v[:, :, 5 + ev:12 + ev],
                    in0=Wf_v[:, :, 5 + ev:12 + ev],
                    in1=mf[:].rearrange("p (tg k) -> p tg k", k=K), op=OP.add)
                mc = mfpool.tile([P, NT * GK], f16, tag="mc")
                nc.vector.scalar_tensor_tensor(
                    out=mc[:], in0=e_sb[:], scalar=float(ev), in1=wgtc_sb[:],
                    op0=OP.is_equal, op1=OP.mult)
                nc.vector.tensor_tensor(
                    out=Wc_v[:, :, 6 + ev:13 + ev],
                    in0=Wc_v[:, :, 6 + ev:13 + ev],
                    in1=mc[:].rearrange("p (tg k) -> p tg k", k=K), op=OP.add)
        nc.vector.tensor_tensor(out=Wf_sb[:], in0=Wf_sb[:], in1=Wc_sb[:],
                                op=OP.add)

        # ---------------- W -> D-plane scatter (Act ring) -----------------
        Wv = Wf_sb[:].rearrange("p (t g j) -> p t g j", g=G, j=J)
        for g in range(G):
            dst = _ap(Dpls[g][:], [[161, P], [23040, NT], [1, J]], 2560)
            nc.scalar.dma_start(out=dst, in_=Wv[:, :, g, :])
            dst2 = _ap(Dpls[g][:], [[161, 16], [23040, NT], [1, J]],
                       4992 + 161 * 112)
            nc.scalar.dma_start(out=dst2, in_=Wv[112:128, :, g, :])

        # ---------------- band matmuls + y projection ---------------------
        outT_sb = outT_pool.tile([P, G, LCH], f16)
        with (tc.tile_pool(name="band", bufs=2) as bpool,
              tc.tile_pool(name="pband", bufs=8, space="PSUM") as pbp):
            for g in range(G):
                B16 = bpool.tile([P, NB * NSPAN], f16, tag="bimg")
                nc.sync.dma_start(
                    out=B16[:],
                    in_=_ap(Dpls[g][:], [[COLPAD, NB * NSPAN], [1, P]], 0),
                    transpose=True)
                pbs = []
                for c in range(4):
                    pb = pbp.tile([P, 512], f32, tag="pband")
                    nc.tensor.matmul(out=pb[:], lhsT=z1_16[:],
                                     rhs=zrow_16[:], start=True, stop=False)
                    pbs.append(pb)
                for b in range(NB):
                    kb = 128 if b < 16 else 32
                    for (c, f0, f1, col0) in by_b[b]:
                        nc.tensor.matmul(
                            out=pbs[c][:, col0:col0 + (f1 - f0)],
                            lhsT=xp16[:kb, b, 128 * g:128 * g + 128],
                            rhs=B16[:kb, 144 * b + f0:144 * b + f1],
                            start=False,
                            stop=(last_for_chunk[c] == (b, f0)))
                for c in range(4):
                    nc.scalar.activation(
                        out=outT_sb[:, g, 512 * c:512 * c + 512],
                        in_=pbs[c][:], func=AF.Copy)

        # ---------------- y projection ------------------------------------
        with (tc.tile_pool(name="y", bufs=3) as ypool,
              tc.tile_pool(name="py", bufs=4, space="PSUM") as pyp):
            for c in range(4):
                for m in range(CC):
                    py = pyp.tile([P, 512], f32, tag="py")
                    for k in range(CC):
                        nc.tensor.matmul(
                            out=py[:],
                            lhsT=w_out_sb[:, k, 128 * m:128 * m + 128],
                            rhs=outT_sb[:, k, 512 * c:512 * c + 512],
                            start=(k == 0), stop=(k == CC - 1))
                    ysb = ypool.tile([P, 512], f32, tag="ysb")
                    nc.scalar.activation(out=ysb[:], in_=py[:],
                                         func=AF.Identity,
                                         bias=b_out_col(m),
                                         scale=1.0)
                    ydst = _ap(yT[:], [[LCH, P], [1, 512]],
                               128 * m * LCH + 512 * c)
                    yeng = nc.sync if (c * 4 + m) % 2 == 0 else nc.scalar
                    yeng.dma_start(out=ydst, in_=ysb[:])
    return nc


# ---------------- host-side helpers ----------------

def make_core_inputs(inputs, core):
    """Build the per-core input dict from the full problem inputs."""
    n, h = core // 2, core % 2
    start = h * LCH
    x = np.asarray(inputs["x"], np.float32)
    xpad = np.zeros((L + 2 * HALO, C), np.float32)
    xpad[HALO:HALO + L] = x[n]
    xT = np.ascontiguousarray(xpad[start:start + LLOC].T)

    def cmaj(a):  # [C] -> [128, CC] with c = cc*128 + p
        return np.ascontiguousarray(np.asarray(a, np.float32).reshape(CC, P).T)

    dw = np.asarray(inputs["dw_w"], np.float32)[:, 0, :]   # [C, 3]
    dwdiag = np.zeros((P, 12, P), np.float32)
    rng = np.arange(P)
    for cc in range(CC):
        for tap in range(3):
            dwdiag[rng, 3 * cc + tap, rng] = dw[cc * P + rng, tap]

    pos = start + np.arange(LCH)
    kk = np.arange(K)
    pos_ptk = pos.reshape(NT, P).T[:, :, None, None]       # [p, t, 1, 1]
    ones = np.ones((P, NT, G, K), np.float32)
    vlo = (3 - kk[None, None, None, :] - pos_ptk) * ones
    vhi = (L + 2 - kk[None, None, None, :] - pos_ptk) * ones

    f = np.float32
    h16 = np.float16
    small4v = np.concatenate(
        [cmaj(inputs["dw_b"]), cmaj(inputs["ln_g"]),
         cmaj(inputs["ln_b"]), cmaj(inputs["b_out"])], 1)
    vlohiv = np.concatenate(
        [vlo.reshape(P, NT * GK), vhi.reshape(P, NT * GK)], 1)
    return {
        "xT": xT.astype(h16),
        "w_inT": np.ascontiguousarray(
            np.asarray(inputs["w_in"]).T).astype(h16),
        "b_in": np.asarray(inputs["b_in"]).reshape(1, C).astype(h16),
        "dwdiag": np.ascontiguousarray(
            dwdiag.reshape(P, 12 * P)).astype(h16),
        "small4": np.ascontiguousarray(small4v).astype(f),
        "w_omT16": np.ascontiguousarray(np.concatenate(
            [np.asarray(inputs["w_off"]).T, np.asarray(inputs["w_mask"]).T],
            1)).astype(h16),
        "b_om16": np.concatenate([np.asarray(inputs["b_off"]),
                                  np.asarray(inputs["b_mask"])]).reshape(
                                      1, 2 * GK).astype(h16),
        "w_outT16": np.ascontiguousarray(
            np.asarray(inputs["w_out"]).T).astype(h16),
        "vlohi": np.ascontiguousarray(vlohiv).astype(f),
        "ones_cb": _bf16_full((P, 1), 1.0 / C),
    }


def _bf16_full(shape, val):
    import ml_dtypes
    return np.full(shape, val, ml_dtypes.bfloat16)


def assemble(results):
    """results: list of 8 dicts with 'yT' [C, LCH] -> full [4, L, C]."""
    out = np.zeros((4, L, C), np.float32)
    for core in range(8):
        n, h = core // 2, core % 2
        out[n, h * LCH:(h + 1) * LCH] = results[core]["yT"].T
    return out


_NC_CACHE = {}


def kernel(**inputs):
    """Full-problem entry point. inputs keyed as in setup_inputs()."""
    from concourse.bass_utils import run_bass_kernel_spmd
    if "nc" not in _NC_CACHE:
        _NC_CACHE["nc"] = build_nc()
    nc = _NC_CACHE["nc"]
    in_maps = [make_core_inputs(inputs, core) for core in range(8)]
    res = run_bass_kernel_spmd(nc, in_maps, core_ids=list(range(8)))
    return assemble(res.results)


# revision 39
# speedup vs baseline: 1.2371x; 1.0754x over previous
"""Self-contained TRN2 Bass kernel for nn_DeformConv1d_84739704750225.

kernel(**inputs) takes the FULL unsharded inputs (as produced by
setup_inputs()) and returns the FULL [4, 4096, 512] float32 output.

Internally: data-parallel over (sample, length-half) -> 8 NeuronCores via
run_bass_kernel_spmd. The deformable gather is reformulated as banded
matmuls: per-position window weights W[l, g, j] (j in [0,17)) are scattered
to DRAM as a single fp16 "B-image" plane in the exact [block, span, row]
layout the TensorEngine needs, loaded back with a transposing DMA, and
contracted against fp16 x_proj in one pass. The depthwise conv runs on the
TensorEngine via diagonal weight matrices; LN stats use ones-matmul
reductions; offset/mask nets run in fp16. DMA traffic is split across the
SP and Activation HWDGE rings so the W scatter overlaps the x_proj matmuls.
"""
import sys
sys.path.insert(0, "/opt/trn_rl_repo")
import numpy as np
"""Workarounds for this walrus build's 1-sync-wait-per-instruction limit:

1. TileContext tail drain: put global-clock waits on single-wait SP nops.
2. General post-pass after Tile lowering: any instruction carrying more than
   one sem wait gets preceding same-engine NoOps, one wait each.
"""
import concourse.tile as tile
import concourse.mybir as mybir
from concourse.vector_clock import ScopedClock

MAXW = 1


def _drain_and_barrier(self, tick_clock, wait_clock):
    nc = self.nc
    probe = nc.sync.nop(nofuse=True, hint="tail_wait")
    wait_clock.add_sem_waits(probe.ins, ScopedClock({None: tick_clock.global_clock}))
    waits = list(probe.ins.sync_info.on_wait)
    probe.ins.sync_info.on_wait = waits[:MAXW]
    rest = waits[MAXW:]
    while rest:
        n2 = nc.sync.nop(nofuse=True, hint="tail_wait")
        n2.ins.sync_info = mybir.SyncInfo(on_wait=rest[:MAXW], on_update=[])
        rest = rest[MAXW:]
    nc.sync.drain()
    nc.all_engine_barrier()
    popped = nc._tile_sem_poison_stack.pop()
    assert popped is self._sem_poison
    nc.clear_and_free_semaphores(list(self.sems.allocated().values()))
    nc.all_engine_barrier()


def split_excess_waits(nc, maxw=MAXW):
    """Move all but `maxw` sem-waits of each instruction onto preceding
    same-engine NoOps (program order preserved, so semantics unchanged)."""
    nsplit = 0
    for f in nc.m.functions:
        for blk in f.blocks:
            il = blk.instructions
            i = 0
            while i < len(il):
                inst = il[i]
                si = getattr(inst, "sync_info", None)
                ow = list(si.on_wait) if si is not None else []
                if len(ow) > maxw:
                    si.on_wait = ow[len(ow) - maxw:]
                    extra = ow[:len(ow) - maxw]
                    for j, w in enumerate(extra):
                        n = mybir.InstNoOp(name=f"{inst.name}-ws{j}", ins=[],
                                           outs=[])
                        n.engine = inst.engine
                        n.sync_info = mybir.SyncInfo(on_wait=[w], on_update=[])
                        try:
                            nc.register_instruction(n, overwrite=True)
                        except TypeError:
                            nc.register_instruction(n)
                        il.insert(i, n)
                        i += 1
                    nsplit += 1
                i += 1
    return nsplit


_orig_sched = tile.TileContext.schedule_and_allocate


def _patched_sched(self):
    res = _orig_sched(self)
    split_excess_waits(self.nc)
    return res


tile.TileContext._drain_and_barrier = _drain_and_barrier
tile.TileContext.schedule_and_allocate = _patched_sched



import numpy as np
from contextlib import ExitStack

import bass_rust
import concourse.bass as bass
import concourse.mybir as mybir
import concourse.tile as tile

P = 128
C = 512
CC = 4            # c chunks
G = 4
K = 7
GK = G * K        # 28
J = 17            # band window
L = 4096
LCH = 2048
HALO = 64
LLOC = LCH + 2 * HALO   # 2176
NT = 16           # out l-tiles of 128
NB = 17           # band blocks (= xp tiles), last has 32 rows
NSPAN = 144
COLPAD = 160            # D-plane row stride (128 data + 32 guard cols)
DG = 2448 * COLPAD      # per-g D words
MAGIC = 12582912.0      # 1.5 * 2^23
LN_EPS = 1e-5

f32 = mybir.dt.float32
f32r = mybir.dt.float32r
bf16 = mybir.dt.bfloat16
f16 = mybir.dt.float16
AF = mybir.ActivationFunctionType
OP = mybir.AluOpType


def _ap(t_ap, pairs, offset):
    """Custom access pattern over a tensor's base AP."""
    a = t_ap.copy()
    a.ap = bass_rust.VecI64Pair([list(p) for p in pairs])
    a.offset = offset
    return a


def band_pieces():
    """Per 512-chunk: list of (b, f0, f1, col0). Block b out-span
    l in [128b-16, 128b+128) clipped to [0, LCH), split at 512 boundaries."""
    per_chunk = [[] for _ in range(4)]
    for b in range(NB):
        lo = max(0, 128 * b - 16)
        hi = min(LCH, 128 * b + 128)
        s = lo
        while s < hi:
            e = min(hi, (s // 512 + 1) * 512)
            c = s // 512
            per_chunk[c].append((b, s - (128 * b - 16), e - (128 * b - 16),
                                 s - 512 * c))
            s = e
    return per_chunk


def build_nc():
    nc = bass.Bass()

    def inp(name, shape, dt=f32):
        return nc.dram_tensor(name, shape, dt, kind="ExternalInput")

    xT = inp("xT", [C, LLOC], f16)
    w_inT = inp("w_inT", [C, C], f16)     # rows c (contract), cols c_out
    b_in = inp("b_in", [1, C], f16)
    dwdiag = inp("dwdiag", [P, 12 * P], f16)  # 12 diag blocks (cc, tap)
    small4 = inp("small4", [P, 4 * CC])   # [dwb | lng | lnb | b_out] cmaj
    w_omT16 = inp("w_omT16", [C, 2 * GK], f16)  # cols: [off 28 | mask 28]
    b_om16 = inp("b_om16", [1, 2 * GK], f16)    # [b_off | b_mask]
    w_outT16 = inp("w_outT16", [C, C], f16)
    vlohi = inp("vlohi", [P, 2 * NT * GK])  # [p, (lo/hi, t, g, k)]
    ones_cb = inp("ones_cb", [P, 1], bf16)    # 1/512 (for bf16 reductions)
    yT = nc.dram_tensor("yT", [C, LCH], f32, kind="ExternalOutput")

    pieces = band_pieces()
    by_b = [[] for _ in range(NB)]
    for c in range(4):
        for (b, f0, f1, col0) in pieces[c]:
            by_b[b].append((c, f0, f1, col0))
    last_for_chunk = {}
    for b in range(NB):
        for (c, f0, f1, col0) in by_b[b]:
            last_for_chunk[c] = (b, f0)

    with tile.TileContext(nc) as tc, ExitStack() as ctx:
        cpool = ctx.enter_context(tc.tile_pool(name="consts", bufs=1))
        dram = ctx.enter_context(tc.tile_pool(name="dram", bufs=1, space="DRAM"))
        work = ctx.enter_context(tc.tile_pool(name="work", bufs=1))
        tmp2k = ctx.enter_context(tc.tile_pool(name="tmp2k", bufs=2))

        # kernel-lifetime data pools (entered before any scoped pool)
        xdw_pool = ctx.enter_context(tc.tile_pool(name="xdw", bufs=1))
        xdw16_pool = ctx.enter_context(tc.tile_pool(name="xdw16", bufs=1))
        xp_pool = ctx.enter_context(tc.tile_pool(name="xp", bufs=1))
        outT_pool = ctx.enter_context(tc.tile_pool(name="outT", bufs=1))

        # ---------------- input DMAs --------------------------------------
        # SP ring: dwdiag, then xT in 4 column chunks so the conv can start
        # as soon as the first windows land. Everything else rides the Act
        # HWDGE ring in parallel.
        xT_cm = tc.tile_pool(name="xT", bufs=1)
        xT_pool = xT_cm.__enter__()
        xT_sb = xT_pool.tile([P, CC, LLOC], f16)
        dwd_cm = tc.tile_pool(name="dwd", bufs=1)
        dwd_pool = dwd_cm.__enter__()
        dwdiag_sb = dwd_pool.tile([P, 12, P], f16)
        nc.sync.dma_start(out=dwdiag_sb[:], in_=dwdiag[:])
        XCOLS = [(0, 640), (640, 1152), (1152, 1664), (1664, 2176)]
        for c0, c1 in XCOLS:
            src = _ap(xT[:], [[LLOC, P], [P * LLOC, CC], [1, c1 - c0]], c0)
            nc.sync.dma_start(out=xT_sb[:, :, c0:c1], in_=src)

        def load_plain(shape, src, tag, dt=f32):
            t = cpool.tile(shape, dt, tag=tag)
            nc.gpsimd.dma_start(out=t[:], in_=src[:])
            return t

        def load_cmaj(dst, src, ncols):
            # src [C, ncols] -> dst [128, CC, ncols] ; c = cc*128 + p
            src_ap = _ap(src[:], [[ncols, P], [P * ncols, CC], [1, ncols]], 0)
            nc.gpsimd.dma_start(out=dst[:], in_=src_ap)

        small_sb = load_plain([P, 4 * CC], small4, "small4")
        dwb_col = lambda k: small_sb[:, 0 * CC + k:0 * CC + k + 1]
        lng_col = lambda k: small_sb[:, 1 * CC + k:1 * CC + k + 1]
        lnb_col = lambda k: small_sb[:, 2 * CC + k:2 * CC + k + 1]
        b_out_col = lambda m: small_sb[:, 3 * CC + m:3 * CC + m + 1]
        ones_bf_sb = load_plain([P, 1], ones_cb, "ones_cb", bf16)
        vlohi_sb = load_plain([P, 2 * NT * GK], vlohi, "vlohi")
        w_om_sb = cpool.tile([P, CC, 2 * GK], f16)
        load_cmaj(w_om_sb, w_omT16, 2 * GK)
        b_om_sb = load_plain([1, 2 * GK], b_om16, "b_om", f16)
        b_in_sb = load_plain([1, C], b_in, "b_in", f16)
        w_in_sb = cpool.tile([P, CC, C], f16)
        load_cmaj(w_in_sb, w_inT, C)
        w_out_sb = cpool.tile([P, CC, C], f16)
        load_cmaj(w_out_sb, w_outT16, C)
        eps_sb = cpool.tile([1, 1], f32)
        nc.gpsimd.memset(eps_sb[:], LN_EPS)
        one1_16 = cpool.tile([1, P], f16)
        nc.gpsimd.memset(one1_16[:], 1.0)
        z1_16 = cpool.tile([1, P], f16)
        nc.gpsimd.memset(z1_16[:], 0.0)
        zrow_16 = cpool.tile([1, C], f16)
        nc.gpsimd.memset(zrow_16[:], 0.0)

        # ---------------- D plane zero (SP ring) --------------------------
        Dpls = [dram.tile([DG], f16, name="dpl%d" % g, tag="dpl%d" % g)
                for g in range(G)]
        with tc.tile_pool(name="zero", bufs=1) as zpool:
            zt = zpool.tile([P, 3060], f16)
            nc.gpsimd.memset(zt[:], 0.0)
            for g in range(G):
                dst = _ap(Dpls[g][:], [[3060, P], [1, 3060]], 0)
                nc.sync.dma_start(out=dst, in_=zt[:])

        # ---------------- depthwise conv (k=3) on PE ----------------------
        xdw_sb = xdw_pool.tile([P, CC, LCH], bf16)
        with tc.tile_pool(name="psc", bufs=4, space="PSUM") as psc:
            for lc in range(4):
                for k in range(CC):
                    ps = psc.tile([P, 512], f32, tag="psc")
                    for tap in range(3):
                        nc.tensor.matmul(
                            out=ps[:],
                            lhsT=dwdiag_sb[:, 3 * k + tap, :],
                            rhs=xT_sb[:, k, 63 + tap + 512 * lc:
                                      63 + tap + 512 * lc + 512],
                            start=(tap == 0), stop=(tap == 2))
                    nc.scalar.activation(
                        out=xdw_sb[:, k, 512 * lc:512 * lc + 512], in_=ps[:],
                        func=AF.Identity, bias=dwb_col(k), scale=1.0)

        dwd_cm.__exit__(None, None, None)

        # ---------------- LN stats ----------------------------------------
        anorm_cm = tc.tile_pool(name="anorm", bufs=1)
        anorm = anorm_cm.__enter__()
        a_sb = anorm.tile([1, LCH], f16)    # 1/sd
        bn_sb = anorm.tile([1, LCH], f16)   # -mu/sd
        murow = anorm.tile([1, LCH], f32)
        varow = anorm.tile([1, LCH], f32)
        with (tc.tile_pool(name="pst", bufs=2, space="PSUM") as pst,
              tc.tile_pool(name="sqp", bufs=2) as sqp,
              tc.tile_pool(name="smallp", bufs=1) as smallp):
            for lc in range(4):
                sl = slice(512 * lc, 512 * lc + 512)
                pm = pst.tile([1, 512], f32, tag="pmu")
                for k in range(CC):
                    nc.tensor.matmul(
                        out=pm[:], lhsT=ones_bf_sb[:],
                        rhs=xdw_sb[:, k, sl],
                        start=(k == 0), stop=(k == CC - 1))
                pq = pst.tile([1, 512], f32, tag="psq")
                for k in range(CC):
                    sq = sqp.tile([P, 512], bf16, tag="sq")
                    nc.vector.tensor_tensor(out=sq[:], in0=xdw_sb[:, k, sl],
                                            in1=xdw_sb[:, k, sl], op=OP.mult)
                    nc.tensor.matmul(
                        out=pq[:], lhsT=ones_bf_sb[:],
                        rhs=sq[:],
                        start=(k == 0), stop=(k == CC - 1))
                nc.vector.tensor_copy(out=murow[:, sl], in_=pm[:])
                t1 = smallp.tile([1, 512], f32, tag="st1")
                nc.vector.tensor_tensor(out=t1[:], in0=murow[:, sl],
                                        in1=murow[:, sl], op=OP.mult)
                nc.vector.tensor_tensor(out=varow[:, sl], in0=pq[:],
                                        in1=t1[:], op=OP.subtract)

        xp16 = xp_pool.tile([P, NB, C], f16)
        psx_cm = tc.tile_pool(name="psx", bufs=3, space="PSUM")
        psx = psx_cm.__enter__()

        def xproj_tile(mt):
            M = 128 if mt < 16 else 32
            ps = psx.tile([P, C], f32, tag="psx")
            for k in range(CC):
                nc.tensor.matmul(
                    out=ps[:M, :],
                    lhsT=xT_sb[:, k, 56 + 128 * mt:56 + 128 * mt + M],
                    rhs=w_in_sb[:, k, :],
                    start=(k == 0), stop=False)
            nc.tensor.matmul(
                out=ps[:M, :], lhsT=one1_16[:1, :M],
                rhs=b_in_sb[:], start=False, stop=True)
            nc.scalar.activation(out=xp16[:M, mt, :], in_=ps[:M, :],
                                 func=AF.Copy)

        for mt in range(3):
            xproj_tile(mt)

        t3r = anorm.tile([1, LCH], f32)
        nc.scalar.activation(out=t3r[:], in_=varow[:], func=AF.Ln,
                             bias=eps_sb[:])
        t4r = varow
        nc.scalar.activation(out=t4r[:], in_=t3r[:], func=AF.Exp,
                             scale=-0.5)
        nc.vector.tensor_copy(out=a_sb[:], in_=t4r[:])
        nc.vector.scalar_tensor_tensor(
            out=bn_sb[:], in0=murow[:], scalar=-1.0,
            in1=t4r[:], op0=OP.mult, op1=OP.mult)

        # ---------------- normalize + GELU --------------------------------
        xdw16 = xdw16_pool.tile([P, CC, LCH], f16)
        a_rep = anorm.tile([P, LCH], bf16)
        bn_rep = anorm.tile([P, LCH], bf16)
        with tc.tile_pool(name="prep", bufs=2, space="PSUM") as prep:
            for lc in range(4):
                sl = slice(512 * lc, 512 * lc + 512)
                pa = prep.tile([P, 512], f32, tag="pa")
                nc.tensor.matmul(out=pa[:], lhsT=one1_16[:],
                                 rhs=a_sb[:, sl],
                                 start=True, stop=True)
                nc.scalar.activation(out=a_rep[:, sl], in_=pa[:], func=AF.Copy)
                pb = prep.tile([P, 512], f32, tag="pb")
                nc.tensor.matmul(out=pb[:], lhsT=one1_16[:],
                                 rhs=bn_sb[:, sl],
                                 start=True, stop=True)
                nc.scalar.activation(out=bn_rep[:, sl], in_=pb[:], func=AF.Copy)

        for mt in range(3, 8):
            xproj_tile(mt)

        for k in range(CC):
            t1 = tmp2k.tile([P, LCH], bf16, tag="t2k")
            nc.vector.tensor_tensor(
                out=t1[:], in0=xdw_sb[:, k, :], in1=a_rep[:], op=OP.mult)
            t2 = tmp2k.tile([P, LCH], bf16, tag="t2k")
            nc.vector.tensor_tensor(
                out=t2[:], in0=t1[:], in1=bn_rep[:], op=OP.add)
            nc.scalar.activation(out=xdw16[:, k, :], in_=t2[:], func=AF.Gelu,
                                 scale=lng_col(k), bias=lnb_col(k))

        # ---------------- offset/mask nets (fp16) -------------------------
        off_sb = work.tile([P, NT * GK], f32)    # [p, (t, g, k)]
        en_sb = work.tile([P, NT * GK], f32)
        with tc.tile_pool(name="pom", bufs=2, space="PSUM") as pomp:
            for t in range(NT):
                po = pomp.tile([P, 2 * GK], f32, tag="pom")
                for k in range(CC):
                    nc.tensor.matmul(
                        out=po[:],
                        lhsT=xdw16[:, k, 128 * t:128 * t + 128],
                        rhs=w_om_sb[:, k, :],
                        start=(k == 0), stop=False)
                nc.tensor.matmul(
                    out=po[:], lhsT=one1_16[:],
                    rhs=b_om_sb[:], start=False, stop=True)
                nc.vector.tensor_scalar_mul(
                    out=off_sb[:, GK * t:GK * (t + 1)], in0=po[:, 0:GK],
                    scalar1=2.0)
                nc.scalar.activation(out=en_sb[:, GK * t:GK * (t + 1)],
                                     in_=po[:, GK:2 * GK], func=AF.Exp)
        for mt in range(8, NB):
            xproj_tile(mt)
        psx_cm.__exit__(None, None, None)
        anorm_cm.__exit__(None, None, None)
        xT_cm.__exit__(None, None, None)
        # softmax over k
        red_sb = work.tile([P, NT * G], f32)
        en_v = en_sb[:].rearrange("p (tg k) -> p tg k", k=K)
        nc.vector.tensor_reduce(out=red_sb[:], in_=en_v,
                                axis=mybir.AxisListType.X, op=OP.add)
        rln_sb = work.tile([P, NT * G], f32)
        nc.scalar.activation(out=rln_sb[:], in_=red_sb[:], func=AF.Ln)
        rec_sb = work.tile([P, NT * G], f32)
        nc.scalar.activation(out=rec_sb[:], in_=rln_sb[:], func=AF.Exp,
                             scale=-1.0)
        mask_sb = work.tile([P, NT * GK], f16)
        rec_rep = rec_sb[:].unsqueeze(2).broadcast_to([P, NT * G, K])
        nc.vector.tensor_tensor(
            out=mask_sb[:].rearrange("p (tg k) -> p tg k", k=K),
            in0=en_v, in1=rec_rep, op=OP.mult)

        # ---------------- W math (f16 on the DVE for the 2x mode) ---------
        e_sb = work.tile([P, NT * GK], f32)
        nc.vector.tensor_scalar(out=e_sb[:], in0=off_sb[:], scalar1=MAGIC,
                                scalar2=MAGIC, op0=OP.add, op1=OP.subtract)
        gt_sb = work.tile([P, NT * GK], f32)
        nc.vector.tensor_tensor(out=gt_sb[:], in0=e_sb[:], in1=off_sb[:],
                                op=OP.is_gt)
        nc.vector.tensor_tensor(out=e_sb[:], in0=e_sb[:], in1=gt_sb[:],
                                op=OP.subtract)
        e16_sb = work.tile([P, NT * GK], f16)
        nc.vector.tensor_copy(out=e16_sb[:], in_=e_sb[:])
        frac_sb = work.tile([P, NT * GK], f16)
        nc.vector.tensor_tensor(out=frac_sb[:], in0=off_sb[:], in1=e_sb[:],
                                op=OP.subtract)
        ta_sb = work.tile([P, NT * GK], f16)
        nc.vector.tensor_tensor(out=ta_sb[:], in0=off_sb[:],
                                in1=vlohi_sb[:, :NT * GK], op=OP.is_ge)
        tb_sb = work.tile([P, NT * GK], f16)
        nc.vector.tensor_tensor(out=tb_sb[:], in0=off_sb[:],
                                in1=vlohi_sb[:, NT * GK:], op=OP.is_le)
        nc.vector.tensor_tensor(out=ta_sb[:], in0=ta_sb[:], in1=tb_sb[:],
                                op=OP.mult)
        vm_sb = tb_sb
        nc.vector.tensor_tensor(out=vm_sb[:], in0=ta_sb[:], in1=mask_sb[:],
                                op=OP.mult)
        wgtc_sb = ta_sb
        nc.vector.tensor_tensor(out=wgtc_sb[:], in0=frac_sb[:], in1=vm_sb[:],
                                op=OP.mult)
        wgtf_sb = work.tile([P, NT * GK], f16, name="wgtf_sb", tag="wgtf")
        nc.vector.tensor_tensor(out=wgtf_sb[:], in0=vm_sb[:], in1=wgtc_sb[:],
                                op=OP.subtract)

        Wf_sb = work.tile([P, NT * G * J], f16)   # [p, (t, g, j)]
        Wc_sb = work.tile([P, NT * G * J], f16)
        nc.gpsimd.memset(Wf_sb[:], 0.0)
        nc.gpsimd.memset(Wc_sb[:], 0.0)
        with tc.tile_pool(name="mf", bufs=2) as mfpool:
            Wf_v = Wf_sb[:].rearrange("p (tg j) -> p tg j", j=J)
            Wc_v = Wc_sb[:].rearrange("p (tg j) -> p tg j", j=J)
            for ev in range(-4, 4):
                mf = mfpool.tile([P, NT * GK], f16, tag="mf")
                nc.vector.scalar_tensor_tensor(
                    out=mf[:], in0=e16_sb[:], scalar=float(ev), in1=wgtf_sb[:],
                    op0=OP.is_equal, op1=OP.mult)
                nc.vector.tensor_tensor(
                    out=Wf_v[:, :, 5 + ev:12 + ev],
                    in0=Wf_v[:, :, 5 + ev:12 + ev],
                    in1=mf[:].rearrange("p (tg k) -> p tg k", k=K), op=OP.add)
                mc = mfpool.tile([P, NT * GK], f16, tag="mc")
                nc.vector.scalar_tensor_tensor(
                    out=mc[:], in0=e16_sb[:], scalar=float(ev), in1=wgtc_sb[:],
                    op0=OP.is_equal, op1=OP.mult)
                nc.vector.tensor_tensor(
                    out=Wc_v[:, :, 6 + ev:13 + ev],
                    in0=Wc_v[:, :, 6 + ev:13 + ev],
                    in1=mc[:].rearrange("p (tg k) -> p tg k", k=K), op=OP.add)
        nc.vector.tensor_tensor(out=Wf_sb[:], in0=Wf_sb[:], in1=Wc_sb[:],
                                op=OP.add)

        # ---------------- W -> D-plane scatter (Act ring) -----------------
        Wv = Wf_sb[:].rearrange("p (t g j) -> p t g j", g=G, j=J)
        def scatter_g(g):
            seng = nc.sync if g % 2 == 0 else nc.scalar
            dst = _ap(Dpls[g][:], [[161, P], [23040, NT], [1, J]], 2560)
            seng.dma_start(out=dst, in_=Wv[:, :, g, :])
            dst2 = _ap(Dpls[g][:], [[161, 16], [23040, NT], [1, J]],
                       4992 + 161 * 112)
            seng.dma_start(out=dst2, in_=Wv[112:128, :, g, :])

        # ---------------- band matmuls + y projection ---------------------
        outT_sb = outT_pool.tile([P, G, LCH], f16)
        with (tc.tile_pool(name="band", bufs=4) as bpool,
              tc.tile_pool(name="pband", bufs=8, space="PSUM") as pbp):
            # issue scatters (alternating rings) interleaved with the
            # transposing B prefetches (sync ring only) so band g0 can
            # start while later groups still scatter
            B16s = []
            for g in range(G):
                B16s.append(bpool.tile([P, NB * NSPAN], f16, tag="bimg",
                                       name="b16_%d" % g))
            scatter_g(0)
            scatter_g(1)
            nc.sync.dma_start(
                out=B16s[0][:],
                in_=_ap(Dpls[0][:], [[COLPAD, NB * NSPAN], [1, P]], 0),
                transpose=True)
            scatter_g(2)
            scatter_g(3)
            for g in range(1, G):
                nc.sync.dma_start(
                    out=B16s[g][:],
                    in_=_ap(Dpls[g][:], [[COLPAD, NB * NSPAN], [1, P]], 0),
                    transpose=True)
            for g in range(G):
                B16 = B16s[g]
                pbs = []
                for c in range(4):
                    pb = pbp.tile([P, 512], f32, tag="pband")
                    nc.tensor.matmul(out=pb[:], lhsT=z1_16[:],
                                     rhs=zrow_16[:], start=True, stop=False)
                    pbs.append(pb)
                for b in range(NB):
                    kb = 128 if b < 16 else 32
                    for (c, f0, f1, col0) in by_b[b]:
                        nc.tensor.matmul(
                            out=pbs[c][:, col0:col0 + (f1 - f0)],
                            lhsT=xp16[:kb, b, 128 * g:128 * g + 128],
                            rhs=B16[:kb, 144 * b + f0:144 * b + f1],
                            start=False,
                            stop=(last_for_chunk[c] == (b, f0)))
                for c in range(4):
                    nc.vector.tensor_copy(
                        out=outT_sb[:, g, 512 * c:512 * c + 512],
                        in_=pbs[c][:])

        # ---------------- y projection ------------------------------------
        with (tc.tile_pool(name="y", bufs=3) as ypool,
              tc.tile_pool(name="py", bufs=4, space="PSUM") as pyp):
            for c in range(4):
                for m in range(CC):
                    py = pyp.tile([P, 512], f32, tag="py")
                    for k in range(CC):
                        nc.tensor.matmul(
                            out=py[:],
                            lhsT=w_out_sb[:, k, 128 * m:128 * m + 128],
                            rhs=outT_sb[:, k, 512 * c:512 * c + 512],
                            start=(k == 0), stop=(k == CC - 1))
                    ysb = ypool.tile([P, 512], f32, tag="ysb")
                    nc.scalar.activation(out=ysb[:], in_=py[:],
                                         func=AF.Identity,
                                         bias=b_out_col(m),
                                         scale=1.0)
                    ydst = _ap(yT[:], [[LCH, P], [1, 512]],
                               128 * m * LCH + 512 * c)
                    nc.sync.dma_start(out=ydst, in_=ysb[:])
    return nc


# ---------------- host-side helpers ----------------

def make_core_inputs(inputs, core):
    """Build the per-core input dict from the full problem inputs."""
    n, h = core // 2, core % 2
    start = h * LCH
    x = np.asarray(inputs["x"], np.float32)
    xpad = np.zeros((L + 2 * HALO, C), np.float32)
    xpad[HALO:HALO + L] = x[n]
    xT = np.ascontiguousarray(xpad[start:start + LLOC].T)

    def cmaj(a):  # [C] -> [128, CC] with c = cc*128 + p
        return np.ascontiguousarray(np.asarray(a, np.float32).reshape(CC, P).T)

    dw = np.asarray(inputs["dw_w"], np.float32)[:, 0, :]   # [C, 3]
    dwdiag = np.zeros((P, 12, P), np.float32)
    rng = np.arange(P)
    for cc in range(CC):
        for tap in range(3):
            dwdiag[rng, 3 * cc + tap, rng] = dw[cc * P + rng, tap]

    pos = start + np.arange(LCH)
    kk = np.arange(K)
    pos_ptk = pos.reshape(NT, P).T[:, :, None, None]       # [p, t, 1, 1]
    ones = np.ones((P, NT, G, K), np.float32)
    vlo = (3 - kk[None, None, None, :] - pos_ptk) * ones
    vhi = (L + 2 - kk[None, None, None, :] - pos_ptk) * ones

    f = np.float32
    h16 = np.float16
    small4v = np.concatenate(
        [cmaj(inputs["dw_b"]), cmaj(inputs["ln_g"]),
         cmaj(inputs["ln_b"]), cmaj(inputs["b_out"])], 1)
    vlohiv = np.concatenate(
        [vlo.reshape(P, NT * GK), vhi.reshape(P, NT * GK)], 1)
    return {
        "xT": xT.astype(h16),
        "w_inT": np.ascontiguousarray(
            np.asarray(inputs["w_in"]).T).astype(h16),
        "b_in": np.asarray(inputs["b_in"]).reshape(1, C).astype(h16),
        "dwdiag": np.ascontiguousarray(
            dwdiag.reshape(P, 12 * P)).astype(h16),
        "small4": np.ascontiguousarray(small4v).astype(f),
        "w_omT16": np.ascontiguousarray(np.concatenate(
            [np.asarray(inputs["w_off"]).T, np.asarray(inputs["w_mask"]).T],
            1)).astype(h16),
        "b_om16": np.concatenate([np.asarray(inputs["b_off"]),
                                  np.asarray(inputs["b_mask"])]).reshape(
                                      1, 2 * GK).astype(h16),
        "w_outT16": np.ascontiguousarray(
            np.asarray(inputs["w_out"]).T).astype(h16),
        "vlohi": np.ascontiguousarray(vlohiv).astype(f),
        "ones_cb": _bf16_full((P, 1), 1.0 / C),
    }


def _bf16_full(shape, val):
    import ml_dtypes
    return np.full(shape, val, ml_dtypes.bfloat16)


def assemble(results):
    """results: list of 8 dicts with 'yT' [C, LCH] -> full [4, L, C]."""
    out = np.zeros((4, L, C), np.float32)
    for core in range(8):
        n, h = core // 2, core % 2
        out[n, h * LCH:(h + 1) * LCH] = results[core]["yT"].T
    return out


_NC_CACHE = {}


def kernel(**inputs):
    """Full-problem entry point. inputs keyed as in setup_inputs()."""
    from concourse.bass_utils import run_bass_kernel_spmd
    if "nc" not in _NC_CACHE:
        _NC_CACHE["nc"] = build_nc()
    nc = _NC_CACHE["nc"]
    in_maps = [make_core_inputs(inputs, core) for core in range(8)]
    res = run_bass_kernel_spmd(nc, in_maps, core_ids=list(range(8)))
    return assemble(res.results)
